# revision 5
# baseline (speedup 1.0000x reference)
"""Differentiable K-means (VQ codebook) forward on 8 TRN2 NeuronCores.

x: [16, 8192, 64] f32, centroids: [512, 64] f32
out[n] = softmax_k(-(|x_n - c_k|^2)/T) @ C, T = 0.1

Math used here: softmax_k(-(x^2 - 2 x.c + c^2)/T) == softmax_k((2 x.c - c^2)/T)
(the x^2 term cancels).  We compute E[k,n] = exp((2/T) * (x_n . c_k)) and fold
the per-cluster factor W_k = exp(-c_k^2 / T) into the second matmul's moving
operand:  out_aug[n, :] = sum_k E[k,n] * W_k * [C_k | 1]
giving both the unnormalized mixture (cols 0..63) and the softmax denominator
(col 64) in one pass.  out = cols0..63 / col64.

Sharding: data-parallel on the flattened point axis (131072 points -> 8 x
16384), centroids replicated. No cross-core comms.
"""

from contextlib import ExitStack

import numpy as np

import concourse.bass as bass
import concourse.tile as tile
from concourse import bacc, mybir
from concourse._compat import with_exitstack
from concourse.bass_utils import run_bass_kernel_spmd
from concourse.masks import make_identity

N_CORES = 8
N_PTS = 16384  # points per core
K = 512  # clusters
D = 64  # feature dim
TEMP = 0.1
TILE_PTS = 512  # points per inner tile
QS = TILE_PTS // 128  # 4 point-subgroups per tile
KC = K // 128  # 4 cluster chunks

F32 = mybir.dt.float32
F32R = mybir.dt.float32r
BF16 = mybir.dt.bfloat16


@with_exitstack
def _kmeans_body(ctx: ExitStack, tc: tile.TileContext, out_ap, x_ap, c_ap, n_pts):
    nc = tc.nc
    n_tiles = n_pts // TILE_PTS

    # DRAM views: x[(t p q), d] -> [t, p, q*d]; partition p holds QS
    # consecutive points (1KB contiguous per partition -> line-rate DMA).
    x_r = x_ap.rearrange("(t p q) d -> t p (q d)", p=128, q=QS)
    out_r = out_ap.rearrange("(t p q) d -> t p (q d)", p=128, q=QS)

    consts = ctx.enter_context(tc.tile_pool(name="consts", bufs=1))
    small = ctx.enter_context(tc.tile_pool(name="small", bufs=3))
    xin = ctx.enter_context(tc.tile_pool(name="xin", bufs=3))
    xtp = ctx.enter_context(tc.tile_pool(name="xtp", bufs=2))
    epool = ctx.enter_context(tc.tile_pool(name="epool", bufs=2))
    outp = ctx.enter_context(tc.tile_pool(name="outp", bufs=3))
    ps_xt = ctx.enter_context(tc.tile_pool(name="ps_xt", bufs=1, space="PSUM"))
    ps_cr = ctx.enter_context(tc.tile_pool(name="ps_cr", bufs=3, space="PSUM"))
    ps_fin = ctx.enter_context(tc.tile_pool(name="ps_fin", bufs=1, space="PSUM"))

    # ---------------- constants ----------------
    ident = consts.tile([128, 128], F32)
    make_identity(nc, ident)

    # centroids, cluster-chunk layout: partition j of chunk c = cluster 128c+j
    c_sb = consts.tile([128, KC * D], F32)
    nc.sync.dma_start(
        c_sb[:].rearrange("p (c d) -> p c d", d=D),
        c_ap.rearrange("(c p) d -> p c d", p=128),
    )

    # cT [64, 512]: column 128c+j = centroid 128c+j (PE transposes)
    ct_ps = ps_xt.tile([64, K], F32, tag="xt")
    for c in range(KC):
        nc.tensor.transpose(
            ct_ps[:, c * 128 : (c + 1) * 128], c_sb[:, c * D : (c + 1) * D], ident
        )
    ct_sb = consts.tile([64, K], F32R)
    nc.vector.tensor_copy(ct_sb, ct_ps)

    # w = exp(-|c|^2 / T) per cluster  [128, KC]
    csq = small.tile([128, KC * D], F32)
    nc.vector.tensor_mul(csq, c_sb, c_sb)
    csum = small.tile([128, KC], F32)
    nc.vector.reduce_sum(
        csum,
        csq[:].rearrange("p (c d) -> p c d", d=D),
        axis=mybir.AxisListType.X,
    )
    w_sb = consts.tile([128, KC], F32)
    nc.scalar.activation(
        w_sb, csum, mybir.ActivationFunctionType.Exp, scale=-1.0 / TEMP
    )

    # weighted augmented centroids (bf16): cw[:, c*65:...] = [w*C_c | w]
    cw = consts.tile([128, KC * (D + 1)], BF16)
    for c in range(KC):
        nc.vector.tensor_scalar_mul(
            cw[:, c * (D + 1) : c * (D + 1) + D],
            c_sb[:, c * D : (c + 1) * D],
            w_sb[:, c : c + 1],
        )
        nc.vector.tensor_copy(
            cw[:, c * (D + 1) + D : (c + 1) * (D + 1)], w_sb[:, c : c + 1]
        )

    # ---------------- main loop ----------------
    for t in range(n_tiles):
        x_t = xin.tile([128, QS * D], F32)
        nc.sync.dma_start(x_t, x_r[t])

        # xT [64, 512]: column q*128+j = point 4j+q of this tile
        xt_ps = ps_xt.tile([64, TILE_PTS], F32, tag="xt")
        for q in range(QS):
            nc.tensor.transpose(
                xt_ps[:, q * 128 : (q + 1) * 128], x_t[:, q * D : (q + 1) * D], ident
            )
        xt_sb = xtp.tile([64, TILE_PTS], F32R)
        nc.vector.tensor_copy(xt_sb, xt_ps)

        # mm1 (f32r, full rate): cross^T chunks [128 clusters, 512 points]
        # + exp on ScalarE -> E bf16
        e_sb = epool.tile([128, KC * TILE_PTS], BF16)
        for pair in range(KC // 2):
            cr_ps = ps_cr.tile([128, 2 * TILE_PTS], F32, tag="cr")
            for h in range(2):
                c = pair * 2 + h
                nc.tensor.matmul(
                    cr_ps[:, h * TILE_PTS : (h + 1) * TILE_PTS],
                    lhsT=ct_sb[:, c * 128 : (c + 1) * 128],
                    rhs=xt_sb[:],
                    start=True,
                    stop=True,
                )
            nc.scalar.activation(
                e_sb[:, pair * 2 * TILE_PTS : (pair + 1) * 2 * TILE_PTS],
                cr_ps,
                mybir.ActivationFunctionType.Exp,
                scale=2.0 / TEMP,
            )

        # mm2: natural-layout output [128 points, 65] per point-subgroup q,
        # accumulated over cluster chunks. Col 64 = softmax denominator.
        fin_ps = ps_fin.tile([128, QS * (D + 1)], F32, tag="fin")
        for q in range(QS):
            for c in range(KC):
                nc.tensor.matmul(
                    fin_ps[:, q * (D + 1) : (q + 1) * (D + 1)],
                    lhsT=e_sb[
                        :,
                        c * TILE_PTS + q * 128 : c * TILE_PTS + (q + 1) * 128,
                    ],
                    rhs=cw[:, c * (D + 1) : (c + 1) * (D + 1)],
                    start=(c == 0),
                    stop=(c == KC - 1),
                )

        fin3 = fin_ps[:].rearrange("p (q e) -> p q e", e=D + 1)
        inv = small.tile([128, QS], F32)
        nc.vector.reciprocal(inv, fin3[:, :, D])

        o_t = outp.tile([128, QS * D], F32)
        o3 = o_t[:].rearrange("p (q d) -> p q d", d=D)
        nc.vector.tensor_mul(o3, fin3[:, :, 0:D], inv[:].broadcast_to([128, QS, D]))

        nc.sync.dma_start(out_r[t], o_t)


def build_nc(n_pts=N_PTS, debug=False):
    nc = bacc.Bacc("TRN2", target_bir_lowering=False, debug=debug, num_devices=N_CORES)
    x_in = nc.dram_tensor("x", [n_pts, D], F32, kind="ExternalInput").ap()
    c_in = nc.dram_tensor("centroids", [K, D], F32, kind="ExternalInput").ap()
    out = nc.dram_tensor("out", [n_pts, D], F32, kind="ExternalOutput").ap()
    with tile.TileContext(nc) as tc:
        _kmeans_body(tc, out, x_in, c_in, n_pts)
    nc.compile()
    return nc


_NC_CACHE = None


def kernel(x: np.ndarray, centroids: np.ndarray) -> np.ndarray:
    global _NC_CACHE
    orig_shape = x.shape
    xf = np.ascontiguousarray(x.reshape(-1, D).astype(np.float32, copy=False))
    cf = np.ascontiguousarray(centroids.astype(np.float32, copy=False))
    n_total = xf.shape[0]
    assert n_total == N_CORES * N_PTS, n_total

    if _NC_CACHE is None:
        _NC_CACHE = build_nc()
    nc = _NC_CACHE

    in_maps = [
        {"x": np.ascontiguousarray(xf[i * N_PTS : (i + 1) * N_PTS]), "centroids": cf}
        for i in range(N_CORES)
    ]
    res = run_bass_kernel_spmd(nc, in_maps, core_ids=list(range(N_CORES)))
    out = np.concatenate([res.results[i]["out"] for i in range(N_CORES)], axis=0)
    return out.reshape(orig_shape).astype(x.dtype, copy=False)


# revision 6
# speedup vs baseline: 2.0684x; 2.0684x over previous
"""Differentiable K-means (VQ codebook) forward on 8 TRN2 NeuronCores.

x: [16, 8192, 64] f32, centroids: [512, 64] f32
out[n] = softmax_k(-(|x_n - c_k|^2)/T) @ C, T = 0.1

Math: softmax_k(-(x^2 - 2 x.c + c^2)/T) == softmax_k((2 x.c - c^2)/T)
(the x^2 term cancels). We compute E[k,n] = exp((2/T) * (x_n . c_k)) on the
ScalarE and fold the per-cluster factor W_k = exp(-c_k^2/T) into the second
matmul's moving operand: out_aug[n,:] = sum_k E[k,n] * W_k * [C_k | 1], giving
the unnormalized mixture (cols 0..63) and softmax denominator (col 64) in one
pass. out = cols0..63 / col64.

Device dataflow (per core, 16384 points):
- host pre-transposes the x shard to xT [64, 16384] (column-permuted so that
  within each 512-pt tile, col q*128+j = point 4j+q -> both input and output
  DMAs are 1KB-contiguous per partition).
- xT tile is DMA'd twice (rows 0:64 / 64:128) so mm1 can run K=64 matmuls
  row-packed two-at-a-time in PE row groups 0/64 (2x matmul throughput).
- mm1 (float32r, full rate): cross^T chunks [128 clusters, 512 pts] in PSUM.
- exp on ScalarE: E = exp(20*cross) -> bf16 SBUF.
- mm2: lhsT = E slices (bf16, FWL), moving = weighted-augmented centroids
  [128,65] -> natural [128 pts, 65] PSUM accumulated over cluster chunks.
- reciprocal + multiply on VectorE, contiguous DMA out.

Sharding: data-parallel on the flattened point axis (131072 -> 8 x 16384),
centroids replicated. No cross-core comms.
"""

from contextlib import ExitStack

import numpy as np

import concourse.bass as bass
import concourse.tile as tile
from concourse import bacc, mybir
from concourse._compat import with_exitstack
from concourse.bass_utils import run_bass_kernel_spmd

N_CORES = 8
N_PTS = 16384  # points per core
K = 512  # clusters
D = 64  # feature dim
TEMP = 0.1
TILE_PTS = 512  # points per inner tile
QS = TILE_PTS // 128  # 4 point-subgroups per tile
KC = K // 128  # 4 cluster chunks

F32 = mybir.dt.float32
F32R = mybir.dt.float32r
BF16 = mybir.dt.bfloat16


@with_exitstack
def _kmeans_body(ctx: ExitStack, tc: tile.TileContext, out_ap, xt_ap, c_ap, n_pts):
    nc = tc.nc
    n_tiles = n_pts // TILE_PTS

    # out[(t p q), d] -> [t, p, q*d]; partition p holds 4 consecutive points
    # (1KB contiguous per partition). xT columns are host-permuted to match:
    # xT col t*512 + q*128 + j  =  point t*512 + 4j + q.
    out_r = out_ap.rearrange("(t p q) d -> t p (q d)", p=128, q=QS)

    consts = ctx.enter_context(tc.tile_pool(name="consts", bufs=1))
    small = ctx.enter_context(tc.tile_pool(name="small", bufs=3))
    xtp = ctx.enter_context(tc.tile_pool(name="xtp", bufs=3))
    epool = ctx.enter_context(tc.tile_pool(name="epool", bufs=2))
    outp = ctx.enter_context(tc.tile_pool(name="outp", bufs=3))
    ps_cr = ctx.enter_context(tc.tile_pool(name="ps_cr", bufs=3, space="PSUM"))
    ps_fin = ctx.enter_context(tc.tile_pool(name="ps_fin", bufs=2, space="PSUM"))

    # ---------------- constants ----------------
    ident = consts.tile([128, 128], F32)
    from concourse.masks import make_identity

    make_identity(nc, ident)

    # centroids duplicated side-by-side: c2[:, c, 0:64] = c2[:, c, 64:128] =
    # chunk c of C (partition j = cluster 128c+j).
    c2_sb = consts.tile([128, KC, 2, D], F32)
    for h in range(2):
        nc.sync.dma_start(
            c2_sb[:, :, h, :], c_ap.rearrange("(c p) d -> p c d", p=128)
        )

    # ct2 [128, 512]: rows 0:64 and 64:128 both hold cT (transpose of the
    # doubled chunk gives the vertical duplicate for row-packed matmuls).
    ct2_ps = ps_fin.tile([128, K], F32, tag="fin")
    for c in range(KC):
        nc.tensor.transpose(
            ct2_ps[:, c * 128 : (c + 1) * 128],
            c2_sb[:, c, :, :].rearrange("p a b -> p (a b)"),
            ident,
        )
    ct2_sb = consts.tile([128, K], F32R)
    nc.vector.tensor_copy(ct2_sb, ct2_ps)

    # w = exp(-|c|^2/T) per cluster [128, KC]
    csq = small.tile([128, KC * D], F32)
    nc.vector.tensor_mul(
        csq[:].rearrange("p (c d) -> p c d", d=D),
        c2_sb[:, :, 0, :],
        c2_sb[:, :, 0, :],
    )
    csum = small.tile([128, KC], F32)
    nc.vector.reduce_sum(
        csum,
        csq[:].rearrange("p (c d) -> p c d", d=D),
        axis=mybir.AxisListType.X,
    )
    w_sb = consts.tile([128, KC], F32)
    nc.scalar.activation(
        w_sb, csum, mybir.ActivationFunctionType.Exp, scale=-1.0 / TEMP
    )

    # weighted augmented centroids (bf16): cw[:, c*65:...] = [w*C_c | w]
    cw = consts.tile([128, KC * (D + 1)], BF16)
    for c in range(KC):
        nc.vector.tensor_scalar_mul(
            cw[:, c * (D + 1) : c * (D + 1) + D],
            c2_sb[:, c, 0, :],
            w_sb[:, c : c + 1],
        )
        nc.vector.tensor_copy(
            cw[:, c * (D + 1) + D : (c + 1) * (D + 1)], w_sb[:, c : c + 1]
        )

    # ---------------- main loop ----------------
    for t in range(n_tiles):
        # xT tile duplicated vertically for row-packed K=64 matmuls
        xt2 = xtp.tile([128, TILE_PTS], F32R)
        nc.sync.dma_start(xt2[0:64, :], xt_ap[:, t * TILE_PTS : (t + 1) * TILE_PTS])
        nc.sync.dma_start(
            xt2[64:128, :], xt_ap[:, t * TILE_PTS : (t + 1) * TILE_PTS]
        )

        # mm1: 2 pairs of row-packed f32r matmuls -> cross^T chunks, + exp
        e_sb = epool.tile([128, KC * TILE_PTS], BF16)
        for pair in range(KC // 2):
            cr_ps = ps_cr.tile([128, 2 * TILE_PTS], F32, tag="cr")
            for h in range(2):
                c = pair * 2 + h
                nc.tensor.matmul(
                    cr_ps[:, h * TILE_PTS : (h + 1) * TILE_PTS],
                    lhsT=ct2_sb[h * 64 : (h + 1) * 64, c * 128 : (c + 1) * 128],
                    rhs=xt2[h * 64 : (h + 1) * 64, :],
                    start=True,
                    stop=True,
                )
            nc.scalar.activation(
                e_sb[:, pair * 2 * TILE_PTS : (pair + 1) * 2 * TILE_PTS],
                cr_ps,
                mybir.ActivationFunctionType.Exp,
                scale=2.0 / TEMP,
            )

        # mm2: natural-layout [128 pts, 65] per point-subgroup q, accumulated
        # over cluster chunks; col 64 = softmax denominator.
        fin_ps = ps_fin.tile([128, QS * (D + 1)], F32, tag="fin")
        for q in range(QS):
            for c in range(KC):
                nc.tensor.matmul(
                    fin_ps[:, q * (D + 1) : (q + 1) * (D + 1)],
                    lhsT=e_sb[
                        :,
                        c * TILE_PTS + q * 128 : c * TILE_PTS + (q + 1) * 128,
                    ],
                    rhs=cw[:, c * (D + 1) : (c + 1) * (D + 1)],
                    start=(c == 0),
                    stop=(c == KC - 1),
                )

        fin3 = fin_ps[:].rearrange("p (q e) -> p q e", e=D + 1)
        inv = small.tile([128, QS], F32)
        nc.vector.reciprocal(inv, fin3[:, :, D])

        o_t = outp.tile([128, QS * D], F32)
        o3 = o_t[:].rearrange("p (q d) -> p q d", d=D)
        nc.vector.tensor_mul(o3, fin3[:, :, 0:D], inv[:].broadcast_to([128, QS, D]))

        nc.sync.dma_start(out_r[t], o_t)


def build_nc(n_pts=N_PTS, debug=False):
    nc = bacc.Bacc("TRN2", target_bir_lowering=False, debug=debug, num_devices=N_CORES)
    xt_in = nc.dram_tensor("xt", [D, n_pts], F32R, kind="ExternalInput").ap()
    c_in = nc.dram_tensor("centroids", [K, D], F32, kind="ExternalInput").ap()
    out = nc.dram_tensor("out", [n_pts, D], F32, kind="ExternalOutput").ap()
    with tile.TileContext(nc) as tc:
        _kmeans_body(tc, out, xt_in, c_in, n_pts)
    nc.compile()
    return nc


def _host_xt(x_shard: np.ndarray) -> np.ndarray:
    """[n, 64] -> column-permuted transpose [64, n]:
    xT[d, t*512 + q*128 + j] = x[t*512 + 4j + q, d]."""
    n = x_shard.shape[0]
    xs = x_shard.reshape(n // TILE_PTS, 128, QS, D)
    return np.ascontiguousarray(xs.transpose(3, 0, 2, 1).reshape(D, n))


_NC_CACHE = None


def kernel(x: np.ndarray, centroids: np.ndarray) -> np.ndarray:
    global _NC_CACHE
    orig_shape = x.shape
    xf = x.reshape(-1, D).astype(np.float32, copy=False)
    cf = np.ascontiguousarray(centroids.astype(np.float32, copy=False))
    n_total = xf.shape[0]
    assert n_total == N_CORES * N_PTS, n_total

    if _NC_CACHE is None:
        _NC_CACHE = build_nc()
    nc = _NC_CACHE

    in_maps = [
        {"xt": _host_xt(xf[i * N_PTS : (i + 1) * N_PTS]), "centroids": cf}
        for i in range(N_CORES)
    ]
    res = run_bass_kernel_spmd(nc, in_maps, core_ids=list(range(N_CORES)))
    out = np.concatenate([res.results[i]["out"] for i in range(N_CORES)], axis=0)
    return out.reshape(orig_shape).astype(x.dtype, copy=False)


# revision 10
# speedup vs baseline: 2.0701x; 1.0009x over previous
"""Differentiable K-means (VQ codebook) forward on 8 TRN2 NeuronCores.

x: [16, 8192, 64] f32, centroids: [512, 64] f32
out[n] = softmax_k(-(|x_n - c_k|^2)/T) @ C, T = 0.1

Math: softmax_k(-(x^2 - 2 x.c + c^2)/T) == softmax_k((2 x.c - c^2)/T)
(the x^2 term cancels). We compute E[k,n] = exp((2/T) * (x_n . c_k)) on the
ScalarE and fold the per-cluster factor W_k = exp(-c_k^2/T) into the second
matmul's moving operand: out_aug[n,:] = sum_k E[k,n] * W_k * [C_k | 1], giving
the unnormalized mixture (cols 0..63) and softmax denominator (col 64) in one
pass. out = cols0..63 / col64.

Device dataflow (per core, 16384 points):
- host pre-transposes the x shard to xT [64, 16384] (column-permuted so that
  within each 512-pt tile, col q*128+j = point 4j+q -> both input and output
  DMAs are 1KB-contiguous per partition).
- xT tile is DMA'd twice (rows 0:64 / 64:128) so mm1 can run K=64 matmuls
  row-packed two-at-a-time in PE row groups 0/64 (2x matmul throughput).
- mm1 (float32r, full rate): cross^T chunks [128 clusters, 512 pts] in PSUM.
- exp on ScalarE: E = exp(20*cross) -> bf16 SBUF.
- mm2: lhsT = E slices (bf16, FWL), moving = weighted-augmented centroids
  [128,65] -> natural [128 pts, 65] PSUM accumulated over cluster chunks.
- reciprocal + multiply on VectorE, contiguous DMA out.

Sharding: data-parallel on the flattened point axis (131072 -> 8 x 16384),
centroids replicated. No cross-core comms.
"""

from contextlib import ExitStack

import numpy as np

import concourse.bass as bass
import concourse.tile as tile
from concourse import bacc, mybir
from concourse._compat import with_exitstack
from concourse.bass_utils import run_bass_kernel_spmd

N_CORES = 8
N_PTS = 16384  # points per core
K = 512  # clusters
D = 64  # feature dim
TEMP = 0.1
TILE_PTS = 512  # points per inner tile
QS = TILE_PTS // 128  # 4 point-subgroups per tile
KC = K // 128  # 4 cluster chunks

F32 = mybir.dt.float32
F32R = mybir.dt.float32r
BF16 = mybir.dt.bfloat16


@with_exitstack
def _kmeans_body(ctx: ExitStack, tc: tile.TileContext, out_ap, xt_ap, c_ap, n_pts):
    nc = tc.nc
    n_tiles = n_pts // TILE_PTS

    # out[(t p q), d] -> [t, p, q*d]; partition p holds 4 consecutive points
    # (1KB contiguous per partition). xT columns are host-permuted to match:
    # xT col t*512 + q*128 + j  =  point t*512 + 4j + q.
    out_r = out_ap.rearrange("(t p q) d -> t p (q d)", p=128, q=QS)

    consts = ctx.enter_context(tc.tile_pool(name="consts", bufs=1))
    small = ctx.enter_context(tc.tile_pool(name="small", bufs=3))
    xtp = ctx.enter_context(tc.tile_pool(name="xtp", bufs=3))
    epool = ctx.enter_context(tc.tile_pool(name="epool", bufs=2))
    outp = ctx.enter_context(tc.tile_pool(name="outp", bufs=3))
    ps_cr = ctx.enter_context(tc.tile_pool(name="ps_cr", bufs=3, space="PSUM"))
    ps_fin = ctx.enter_context(tc.tile_pool(name="ps_fin", bufs=2, space="PSUM"))

    # ---------------- constants ----------------
    ident = consts.tile([128, 128], F32)
    from concourse.masks import make_identity

    make_identity(nc, ident)

    # centroids duplicated side-by-side: c2[:, c, 0:64] = c2[:, c, 64:128] =
    # chunk c of C (partition j = cluster 128c+j).
    c2_sb = consts.tile([128, KC, 2, D], F32)
    for h in range(2):
        nc.sync.dma_start(
            c2_sb[:, :, h, :], c_ap.rearrange("(c p) d -> p c d", p=128)
        )

    # ct2 [128, 512]: rows 0:64 and 64:128 both hold cT (transpose of the
    # doubled chunk gives the vertical duplicate for row-packed matmuls).
    ct2_ps = ps_fin.tile([128, K], F32, tag="fin")
    for c in range(KC):
        nc.tensor.transpose(
            ct2_ps[:, c * 128 : (c + 1) * 128],
            c2_sb[:, c, :, :].rearrange("p a b -> p (a b)"),
            ident,
        )
    ct2_sb = consts.tile([128, K], F32R)
    nc.vector.tensor_copy(ct2_sb, ct2_ps)

    # w = exp(-|c|^2/T) per cluster [128, KC]
    csq = small.tile([128, KC * D], F32)
    nc.vector.tensor_mul(
        csq[:].rearrange("p (c d) -> p c d", d=D),
        c2_sb[:, :, 0, :],
        c2_sb[:, :, 0, :],
    )
    csum = small.tile([128, KC], F32)
    nc.vector.reduce_sum(
        csum,
        csq[:].rearrange("p (c d) -> p c d", d=D),
        axis=mybir.AxisListType.X,
    )
    w_sb = consts.tile([128, KC], F32)
    nc.scalar.activation(
        w_sb, csum, mybir.ActivationFunctionType.Exp, scale=-1.0 / TEMP
    )

    # weighted augmented centroids (bf16): cw[:, c*65:...] = [w*C_c | w]
    cw = consts.tile([128, KC * (D + 1)], BF16)
    for c in range(KC):
        nc.vector.tensor_scalar_mul(
            cw[:, c * (D + 1) : c * (D + 1) + D],
            c2_sb[:, c, 0, :],
            w_sb[:, c : c + 1],
        )
        nc.vector.tensor_copy(
            cw[:, c * (D + 1) + D : (c + 1) * (D + 1)], w_sb[:, c : c + 1]
        )

    # ---------------- main loop ----------------
    xt2 = None
    o2_t = None
    for t in range(n_tiles):
        # xT loaded two tiles at a time, duplicated vertically (rows 0:64 /
        # 64:128) for row-packed K=64 matmuls
        if t % 2 == 0:
            xt2 = xtp.tile([128, 2 * TILE_PTS], F32R, tag="xt2")
            span = xt_ap[:, t * TILE_PTS : (t + 2) * TILE_PTS]
            nc.sync.dma_start(xt2[0:64, :], span)
            nc.sync.dma_start(xt2[64:128, :], span)
        toff = (t % 2) * TILE_PTS

        # mm1: 2 pairs of row-packed f32r matmuls -> cross^T chunks, + exp
        e_sb = epool.tile([128, KC * TILE_PTS], BF16)
        for pair in range(KC // 2):
            cr_ps = ps_cr.tile([128, 2 * TILE_PTS], F32, tag="cr")
            for h in range(2):
                c = pair * 2 + h
                nc.tensor.matmul(
                    cr_ps[:, h * TILE_PTS : (h + 1) * TILE_PTS],
                    lhsT=ct2_sb[h * 64 : (h + 1) * 64, c * 128 : (c + 1) * 128],
                    rhs=xt2[h * 64 : (h + 1) * 64, toff : toff + TILE_PTS],
                    start=True,
                    stop=True,
                )
            nc.scalar.activation(
                e_sb[:, pair * 2 * TILE_PTS : (pair + 1) * 2 * TILE_PTS],
                cr_ps,
                mybir.ActivationFunctionType.Exp,
                scale=2.0 / TEMP,
            )

        # mm2: natural-layout [128 pts, 65] per point-subgroup q, accumulated
        # over cluster chunks; col 64 = softmax denominator.
        fin_ps = ps_fin.tile([128, QS * (D + 1)], F32, tag="fin")
        for q in range(QS):
            for c in range(KC):
                nc.tensor.matmul(
                    fin_ps[:, q * (D + 1) : (q + 1) * (D + 1)],
                    lhsT=e_sb[
                        :,
                        c * TILE_PTS + q * 128 : c * TILE_PTS + (q + 1) * 128,
                    ],
                    rhs=cw[:, c * (D + 1) : (c + 1) * (D + 1)],
                    start=(c == 0),
                    stop=(c == KC - 1),
                )

        fin3 = fin_ps[:].rearrange("p (q e) -> p q e", e=D + 1)
        inv = small.tile([128, QS], F32)
        nc.vector.reciprocal(inv, fin3[:, :, D])

        # output accumulated two tiles per buffer, DMA'd out via SWDGE
        # (gpsimd) to keep the HWDGE sequencer free for xT loads
        if t % 2 == 0:
            o2_t = outp.tile([128, 2, QS * D], F32, tag="o2")
        o3 = o2_t[:, t % 2, :].rearrange("p (q d) -> p q d", d=D)
        nc.vector.tensor_mul(o3, fin3[:, :, 0:D], inv[:].broadcast_to([128, QS, D]))

        if t % 2 == 1:
            nc.gpsimd.dma_start(
                out_r[t - 1 : t + 1].rearrange("a p n -> p a n"), o2_t
            )


def build_nc(n_pts=N_PTS, debug=False):
    nc = bacc.Bacc("TRN2", target_bir_lowering=False, debug=debug, num_devices=N_CORES)
    xt_in = nc.dram_tensor("xt", [D, n_pts], F32R, kind="ExternalInput").ap()
    c_in = nc.dram_tensor("centroids", [K, D], F32, kind="ExternalInput").ap()
    out = nc.dram_tensor("out", [n_pts, D], F32, kind="ExternalOutput").ap()
    with tile.TileContext(nc) as tc:
        _kmeans_body(tc, out, xt_in, c_in, n_pts)
    nc.compile()
    return nc


def _host_xt(x_shard: np.ndarray) -> np.ndarray:
    """[n, 64] -> column-permuted transpose [64, n]:
    xT[d, t*512 + q*128 + j] = x[t*512 + 4j + q, d]."""
    n = x_shard.shape[0]
    xs = x_shard.reshape(n // TILE_PTS, 128, QS, D)
    return np.ascontiguousarray(xs.transpose(3, 0, 2, 1).reshape(D, n))


_NC_CACHE = None


def kernel(x: np.ndarray, centroids: np.ndarray) -> np.ndarray:
    global _NC_CACHE
    orig_shape = x.shape
    xf = x.reshape(-1, D).astype(np.float32, copy=False)
    cf = np.ascontiguousarray(centroids.astype(np.float32, copy=False))
    n_total = xf.shape[0]
    assert n_total == N_CORES * N_PTS, n_total

    if _NC_CACHE is None:
        _NC_CACHE = build_nc()
    nc = _NC_CACHE

    in_maps = [
        {"xt": _host_xt(xf[i * N_PTS : (i + 1) * N_PTS]), "centroids": cf}
        for i in range(N_CORES)
    ]
    res = run_bass_kernel_spmd(nc, in_maps, core_ids=list(range(N_CORES)))
    out = np.concatenate([res.results[i]["out"] for i in range(N_CORES)], axis=0)
    return out.reshape(orig_shape).astype(x.dtype, copy=False)


# revision 11
# speedup vs baseline: 2.4750x; 1.1956x over previous
"""Differentiable K-means (VQ codebook) forward on 8 TRN2 NeuronCores.

x: [16, 8192, 64] f32, centroids: [512, 64] f32
out[n] = softmax_k(-(|x_n - c_k|^2)/T) @ C, T = 0.1

Math: softmax_k(-(x^2 - 2 x.c + c^2)/T) == softmax_k((2 x.c - c^2)/T)
(the x^2 term cancels). Device computes E[k,n] = exp((2/T) * (x_n . c_k)) on
the ScalarE; the per-cluster factor W_k = exp(-c_k^2/T) is folded into the
second matmul's moving operand: out_aug[n,:] = sum_k E[k,n] * W_k * [C_k | 1],
giving the unnormalized mixture (cols 0..63) and the softmax denominator
(col 64) in one pass. out = cols0..63 / col64.

Device dataflow (per core, 16384 points):
- host pre-transposes the x shard to xT [64, 16384] (column-permuted so that
  within each 512-pt tile, col q*128+j = point 4j+q -> both input and output
  DMAs are 1KB-contiguous per partition). Host also prepares the centroid
  constants (vertically-duplicated cT, bf16 weighted-augmented centroids).
- xT tiles DMA'd twice (rows 0:64 / 64:128) so mm1 runs K=64 f32r matmuls
  row-packed two-at-a-time in PE row groups 0/64 (2x matmul throughput).
- mm1: cross^T chunks [128 clusters, 512 pts] in PSUM -> exp on ScalarE
  (bf16 out) -> mm2 with E slices as stationary (bf16 FWL weight path) and
  [w*C_c | w] as 65-wide moving operand -> natural [128 pts, 65] PSUM.
- reciprocal + multiply on VectorE, contiguous DMA out via SWDGE.

Sharding: data-parallel on the flattened point axis (131072 -> 8 x 16384),
centroids replicated. No cross-core comms.
"""

from contextlib import ExitStack

import ml_dtypes
import numpy as np

import concourse.bass as bass
import concourse.tile as tile
from concourse import bacc, mybir
from concourse._compat import with_exitstack
from concourse.bass_utils import run_bass_kernel_spmd

N_CORES = 8
N_PTS = 16384  # points per core
K = 512  # clusters
D = 64  # feature dim
TEMP = 0.1
TILE_PTS = 512  # points per inner tile
QS = TILE_PTS // 128  # 4 point-subgroups per tile
KC = K // 128  # 4 cluster chunks

F32 = mybir.dt.float32
F32R = mybir.dt.float32r
BF16 = mybir.dt.bfloat16


@with_exitstack
def _kmeans_body(ctx: ExitStack, tc: tile.TileContext, out_ap, xt_ap, ct2_ap, cw_ap, n_pts):
    nc = tc.nc
    n_tiles = n_pts // TILE_PTS

    # out[(t p q), d] -> [t, p, q*d]; partition p holds 4 consecutive points
    # (1KB contiguous per partition). xT columns are host-permuted to match:
    # xT col t*512 + q*128 + j  =  point t*512 + 4j + q.
    out_r = out_ap.rearrange("(t p q) d -> t p (q d)", p=128, q=QS)

    consts = ctx.enter_context(tc.tile_pool(name="consts", bufs=1))
    small = ctx.enter_context(tc.tile_pool(name="small", bufs=3))
    xtp = ctx.enter_context(tc.tile_pool(name="xtp", bufs=3))
    epool = ctx.enter_context(tc.tile_pool(name="epool", bufs=2))
    outp = ctx.enter_context(tc.tile_pool(name="outp", bufs=3))
    ps_cr = ctx.enter_context(tc.tile_pool(name="ps_cr", bufs=3, space="PSUM"))
    ps_fin = ctx.enter_context(tc.tile_pool(name="ps_fin", bufs=2, space="PSUM"))

    # constants, host-precomputed
    ct2_sb = consts.tile([128, K], F32R)
    nc.sync.dma_start(ct2_sb, ct2_ap)
    cw = consts.tile([128, KC * (D + 1)], BF16)
    nc.sync.dma_start(cw, cw_ap)

    def load_xt(t):
        xt2 = xtp.tile([128, 2 * TILE_PTS], F32R, tag="xt2", name=f"xt2_{t}")
        span = xt_ap[:, t * TILE_PTS : (t + 2) * TILE_PTS]
        nc.sync.dma_start(xt2[0:64, :], span)
        nc.sync.dma_start(xt2[64:128, :], span)
        return xt2

    def mm1_exp(t, xt2):
        toff = (t % 2) * TILE_PTS
        e_sb = epool.tile([128, KC * TILE_PTS], BF16, tag="e", name=f"e_{t}")
        for pair in range(KC // 2):
            cr_ps = ps_cr.tile([128, 2 * TILE_PTS], F32, tag="cr", name=f"cr_{t}_{pair}")
            for h in range(2):
                c = pair * 2 + h
                nc.tensor.matmul(
                    cr_ps[:, h * TILE_PTS : (h + 1) * TILE_PTS],
                    lhsT=ct2_sb[h * 64 : (h + 1) * 64, c * 128 : (c + 1) * 128],
                    rhs=xt2[h * 64 : (h + 1) * 64, toff : toff + TILE_PTS],
                    start=True,
                    stop=True,
                )
            nc.scalar.activation(
                e_sb[:, pair * 2 * TILE_PTS : (pair + 1) * 2 * TILE_PTS],
                cr_ps,
                mybir.ActivationFunctionType.Exp,
                scale=2.0 / TEMP,
            )
        return e_sb

    def mm2_norm(t, e_sb, o2_t):
        fin_ps = ps_fin.tile([128, QS * (D + 1)], F32, tag="fin", name=f"fin_{t}")
        for q in range(QS):
            for c in range(KC):
                nc.tensor.matmul(
                    fin_ps[:, q * (D + 1) : (q + 1) * (D + 1)],
                    lhsT=e_sb[
                        :, c * TILE_PTS + q * 128 : c * TILE_PTS + (q + 1) * 128
                    ],
                    rhs=cw[:, c * (D + 1) : (c + 1) * (D + 1)],
                    start=(c == 0),
                    stop=(c == KC - 1),
                )
        fin3 = fin_ps[:].rearrange("p (q e) -> p q e", e=D + 1)
        inv = small.tile([128, QS], F32, tag="inv", name=f"inv_{t}")
        nc.vector.reciprocal(inv, fin3[:, :, D])
        o3 = o2_t[:, t % 2, :].rearrange("p (q d) -> p q d", d=D)
        nc.vector.tensor_mul(o3, fin3[:, :, 0:D], inv[:].broadcast_to([128, QS, D]))

    # main loop, software-pipelined one tile deep: mm1/exp of tile t+1 is
    # emitted before mm2/normalize of tile t so the PE keeps feeding ScalarE.
    xt2 = load_xt(0)
    e_prev = mm1_exp(0, xt2)
    o2_t = None
    for t in range(1, n_tiles + 1):
        if t < n_tiles:
            if t % 2 == 0:
                xt2 = load_xt(t)
            e_cur = mm1_exp(t, xt2)
        if (t - 1) % 2 == 0:
            o2_t = outp.tile([128, 2, QS * D], F32, tag="o2", name=f"o2_{t - 1}")
        mm2_norm(t - 1, e_prev, o2_t)
        if (t - 1) % 2 == 1:
            nc.gpsimd.dma_start(
                out_r[t - 2 : t].rearrange("a p n -> p a n"), o2_t
            )
        if t < n_tiles:
            e_prev = e_cur


def build_nc(n_pts=N_PTS, debug=False):
    nc = bacc.Bacc("TRN2", target_bir_lowering=False, debug=debug, num_devices=N_CORES)
    xt_in = nc.dram_tensor("xt", [D, n_pts], F32R, kind="ExternalInput").ap()
    ct2_in = nc.dram_tensor("ct2", [128, K], F32R, kind="ExternalInput").ap()
    cw_in = nc.dram_tensor("cw", [128, KC * (D + 1)], BF16, kind="ExternalInput").ap()
    out = nc.dram_tensor("out", [n_pts, D], F32, kind="ExternalOutput").ap()
    with tile.TileContext(nc) as tc:
        _kmeans_body(tc, out, xt_in, ct2_in, cw_in, n_pts)
    nc.compile()
    return nc


def _host_xt(x_shard: np.ndarray) -> np.ndarray:
    """[n, 64] -> column-permuted transpose [64, n]:
    xT[d, t*512 + q*128 + j] = x[t*512 + 4j + q, d]."""
    n = x_shard.shape[0]
    xs = x_shard.reshape(n // TILE_PTS, 128, QS, D)
    return np.ascontiguousarray(xs.transpose(3, 0, 2, 1).reshape(D, n))


def _host_consts(centroids: np.ndarray):
    c = centroids.astype(np.float64)
    ct2 = np.concatenate([centroids.T, centroids.T], axis=0).astype(np.float32)
    w = np.exp(-(c * c).sum(-1) / TEMP)  # [K]
    aug = np.concatenate([c * w[:, None], w[:, None]], axis=1)  # [K, 65]
    cw = (
        aug.reshape(KC, 128, D + 1)
        .transpose(1, 0, 2)
        .reshape(128, KC * (D + 1))
        .astype(ml_dtypes.bfloat16)
    )
    return np.ascontiguousarray(ct2), np.ascontiguousarray(cw)


_NC_CACHE = None


def kernel(x: np.ndarray, centroids: np.ndarray) -> np.ndarray:
    global _NC_CACHE
    orig_shape = x.shape
    xf = x.reshape(-1, D).astype(np.float32, copy=False)
    cf = centroids.astype(np.float32, copy=False)
    n_total = xf.shape[0]
    assert n_total == N_CORES * N_PTS, n_total

    if _NC_CACHE is None:
        _NC_CACHE = build_nc()
    nc = _NC_CACHE

    ct2, cw = _host_consts(cf)
    in_maps = [
        {"xt": _host_xt(xf[i * N_PTS : (i + 1) * N_PTS]), "ct2": ct2, "cw": cw}
        for i in range(N_CORES)
    ]
    res = run_bass_kernel_spmd(nc, in_maps, core_ids=list(range(N_CORES)))
    out = np.concatenate([res.results[i]["out"] for i in range(N_CORES)], axis=0)
    return out.reshape(orig_shape).astype(x.dtype, copy=False)
